# revision 20
# baseline (speedup 1.0000x reference)
"""Trainium2 Bass kernel for nn_AdaptiveResBlock (8-core data-parallel).

Reference computation (per batch element b, C=256 channels, T=8192 time):
  for i, dil in enumerate((1, 2, 4)):
      xt = lrelu(x)
      xP, xF = time-gather of xt at round(t -/+ d*dil), zero out-of-range
      xt = WC@xt + WP@xP + WF@xF + biases        (1x1 convs over channels)
      xt = lrelu(xt)
      xt = conv3(xt, WA) + bias
      x = xt + x

Structure used:
  * The time-gather commutes with the 1x1 convs (indices are per-time,
    shared across channels):  WP @ gather(xt) == gather(WP @ xt).
  * Gather offsets are bounded by max(d)*dil <= 64 < 128, so for output
    token block b (128 tokens) the P-gather sources live in token blocks
    {b-1, b} and the F-gather sources in {b, b+1}.
  * The gather itself runs on the TensorEngine as one-hot matmuls in
    token-major space: S[j, t] = (idx(t) == j) built on DVE via is_equal
    against an iota column (relative indices DMA-broadcast from DRAM);
    out-of-range indices get no one-hot bit => free zero masking.
  * Per token block the PSUM accumulation fuses: uC^T (xt-stationary
    matmuls) + one-hot gathers of uP^T / uF^T (+ optional bias rank-1
    matmul).  ACT applies leaky-relu (Prelu, exact on HW) straight from
    PSUM; PE transposes v^T back to channel-major for the 3-tap conv.
  * The u^T token stripe is a rolling window of per-block SBUF tiles
    (B-stage runs a few blocks ahead of the gather stage).
  * lrelu(x) for the next iteration is emitted right behind each tile's
    residual update so the next iteration's matmuls start immediately.

Sharded data-parallel over B=8 across the 8 NeuronCores; (C,C) weights
replicated; per-core relative-gather-index tensors precomputed from d.
"""

import numpy as np
import ml_dtypes
from contextlib import ExitStack

import concourse.bass as bass
import concourse.tile as tile
from concourse import mybir, bacc
from concourse.bass_utils import run_bass_kernel_spmd

F32 = mybir.dt.float32
BF16 = mybir.dt.bfloat16
I16 = mybir.dt.int16
AF = mybir.ActivationFunctionType
OP = mybir.AluOpType

B, C, T_FULL = 8, 256, 8192
DILATIONS = (1, 2, 4)
NITER = len(DILATIONS)
SLOPE = 0.1
INVALID = -512
LAG = 3  # B-stage leads the gather stage by this many token blocks


def build_nc(T=T_FULL, num_devices=8, has_b1=False):
    nT = T // 512            # 512-wide time tiles
    nR = T // 128            # 128-wide token blocks

    nc = bacc.Bacc("TRN2", target_bir_lowering=False, debug=False,
                   num_devices=num_devices)
    x_d = nc.declare_dram_parameter("x", [2, 128, T], F32, isOutput=False)
    wpf_d = nc.declare_dram_parameter("wpf", [NITER, 2, 128, 512], BF16,
                                      isOutput=False)
    wct_d = nc.declare_dram_parameter("wct", [NITER, 2, 128, 256], BF16,
                                      isOutput=False)
    wa_d = nc.declare_dram_parameter("wa", [NITER, 3, 2, 2, 128, 128], BF16,
                                     isOutput=False)
    b3_d = nc.declare_dram_parameter("b3", [NITER, 2, 128, 1], F32,
                                     isOutput=False)
    ixr_d = nc.declare_dram_parameter("ixrel", [NITER, nR, 512], BF16,
                                      isOutput=False)
    iota_d = nc.declare_dram_parameter("iota", [128, 1], F32, isOutput=False)
    id_d = nc.declare_dram_parameter("ident", [128, 128], BF16, isOutput=False)
    if has_b1:
        b1r_d = nc.declare_dram_parameter("b1row", [NITER, 1, 256], BF16,
                                          isOutput=False)
        ones_d = nc.declare_dram_parameter("ones", [1, 128], BF16,
                                           isOutput=False)
    out_d = nc.declare_dram_parameter("out", [2, 128, T], F32, isOutput=True)

    with tile.TileContext(nc) as tc, ExitStack() as ctx:
        xpool = ctx.enter_context(tc.tile_pool(name="xres", bufs=1))
        stp = ctx.enter_context(tc.tile_pool(name="stp", bufs=LAG + 4))
        xtp = ctx.enter_context(tc.tile_pool(name="xtp", bufs=nT))
        vp = ctx.enter_context(tc.tile_pool(name="vp", bufs=nT))
        vtp = ctx.enter_context(tc.tile_pool(name="vtp", bufs=4))
        relp = ctx.enter_context(tc.tile_pool(name="relp", bufs=6))
        sp = ctx.enter_context(tc.tile_pool(name="sp", bufs=6))
        wts = ctx.enter_context(tc.tile_pool(name="wts", bufs=2))
        cst = ctx.enter_context(tc.tile_pool(name="cst", bufs=1))
        pu_ps = ctx.enter_context(tc.tile_pool(name="pu", bufs=2, space="PSUM"))
        pv_ps = ctx.enter_context(tc.tile_pool(name="pv", bufs=2, space="PSUM"))
        pt_ps = ctx.enter_context(tc.tile_pool(name="pt", bufs=4, space="PSUM"))

        def load_weights(i):
            wpf_sb = wts.tile([128, 2, 512], BF16, tag="wpf")
            for cc in range(2):
                nc.sync.dma_start(wpf_sb[:, cc, :], wpf_d[i, cc])
            wct_sb = wts.tile([128, 2, 256], BF16, tag="wct")
            for cc in range(2):
                nc.sync.dma_start(wct_sb[:, cc, :], wct_d[i, cc])
            wa_sb = wts.tile([128, 3, 2, 2, 128], BF16, tag="wa")
            for k in range(3):
                for cc in range(2):
                    for ob in range(2):
                        nc.sync.dma_start(wa_sb[:, k, cc, ob, :],
                                          wa_d[i, k, cc, ob])
            b3_sb = wts.tile([128, 2], F32, tag="b3")
            for ob in range(2):
                nc.sync.dma_start(b3_sb[:, ob:ob + 1], b3_d[i, ob])
            b1r_sb = None
            if has_b1:
                b1r_sb = wts.tile([1, 256], BF16, tag="b1r")
                nc.sync.dma_start(b1r_sb[:, :], b1r_d[i])
            return wpf_sb, wct_sb, wa_sb, b3_sb, b1r_sb

        # Head staging: the first columns of x land in a small dedicated
        # tile with an unambiguous 2-DMA dependency, so iteration 0 starts
        # without waiting for the bulk x load.  Iteration-0 weights go next
        # on the DMA queues, then the rest of x.
        HEAD = min(2048, T)
        x_head = cst.tile([128, 2, HEAD], F32)
        for cb in range(2):
            nc.sync.dma_start(x_head[:, cb, bass.ts(0, 1024)],
                              x_d[cb, :, bass.ts(0, 1024)])
        w0 = load_weights(0)
        for sg in range(1, HEAD // 1024):
            sl = bass.ts(sg, 1024)
            for cb in range(2):
                nc.sync.dma_start(x_head[:, cb, sl], x_d[cb, :, sl])
        iota_sb = cst.tile([128, 1], F32)
        nc.sync.dma_start(iota_sb[:, :], iota_d[:, :])
        ident_sb = cst.tile([128, 128], BF16)
        nc.sync.dma_start(ident_sb[:, :], id_d[:, :])
        if has_b1:
            ones_sb = cst.tile([1, 128], BF16)
            nc.sync.dma_start(ones_sb[:, :], ones_d[:, :])

        # Resident fp32 signal, [128 part, 2 channel-blocks, T].  Columns
        # [0, HEAD) are never read from x_sb in iteration 0 (x_head serves
        # them) and are fully written by iteration 0's residuals.  The bulk
        # load of the rest is emitted interleaved into iteration 0's block
        # loop so it shares the DMA ring fairly with the index broadcasts.
        x_sb = xpool.tile([128, 2, T], F32)
        for sg in range((T - HEAD) // 1024):
            sl = bass.ds(HEAD + sg * 1024, 1024)
            for cb in range(2):
                nc.scalar.dma_start(x_sb[:, cb, sl], x_d[cb, :, sl])

        st_tiles = [None] * nR   # rolling u^T stripe tiles
        xt_tiles = [None] * nT
        xt_next = [None] * nT
        v_tiles = [None] * nT
        vt_tiles = [None] * (nR // 2)
        pv_tiles = [None] * (nR // 2)

        def emit_A(tt, head=False):
            tsl = bass.ts(tt, 512)
            src = x_head[:, :, tsl] if head else x_sb[:, :, tsl]
            xt = xtp.tile([128, 2, 512], BF16, tag="xt")
            nc.vector.scalar_tensor_tensor(
                xt[:, :, :], src, SLOPE, src, OP.mult, OP.max)
            return xt

        def emit_conv3(tt, wa_sb, b3_sb, last, head=False):
            tsl = bass.ts(tt, 512)
            for ob in range(2):
                py = pt_ps.tile([128, 512], F32, tag="pt")
                j = 0
                for k in range(3):
                    for cb in range(2):
                        nc.tensor.matmul(py[:, :], wa_sb[:, k, cb, ob, :],
                                         v_tiles[tt][:, cb, k:k + 512],
                                         start=(j == 0), stop=(j == 5))
                        j += 1
                # residual:  x = (y + b3) + x   (fp32; iter-0 head tiles
                # read the staging copy, write the resident tensor)
                xin = x_head[:, ob, tsl] if head else x_sb[:, ob, tsl]
                nc.vector.scalar_tensor_tensor(
                    x_sb[:, ob, tsl], py[:, :], b3_sb[:, ob:ob + 1],
                    xin, OP.add, OP.add)
            if last:
                for cb in range(2):
                    nc.sync.dma_start(out_d[cb, :, tsl], x_sb[:, cb, tsl])

        for i in range(NITER):
            wpf_sb, wct_sb, wa_sb, b3_sb, b1r_sb = \
                w0 if i == 0 else load_weights(i)

            if i == 0:
                # head tiles up front; later tiles are interleaved into the
                # block loop so x-gated lrelu ops don't sit ahead of the
                # S-builds in the DVE queue.
                for tt in range(HEAD // 512):
                    xt_tiles[tt] = emit_A(tt, head=True)
            else:
                xt_tiles, xt_next = xt_next, [None] * nT

            def emit_B(b):
                tt, off = b // 4, (b % 4) * 128
                ps = pu_ps.tile([128, 512], F32, tag="pu")
                nc.tensor.matmul(ps[:, :], xt_tiles[tt][:, 0, off:off + 128],
                                 wpf_sb[:, 0, :], start=True, stop=False)
                nc.tensor.matmul(ps[:, :], xt_tiles[tt][:, 1, off:off + 128],
                                 wpf_sb[:, 1, :], start=False, stop=True)
                st = stp.tile([128, 512], BF16, tag="st")
                st_tiles[b] = st
                nc.scalar.activation(st[:, :], ps[:, :], AF.Copy)

            def emit_G(b):
                tt, off = b // 4, (b % 4) * 128
                rel = relp.tile([128, 512], BF16, tag="rel")
                nc.sync.dma_start(rel[:, :],
                                  ixr_d[i, b].partition_broadcast(128))
                S = sp.tile([128, 512], BF16, tag="S")
                nc.vector.tensor_scalar(S[:, :], rel[:, :], iota_sb[:, 0:1],
                                        None, OP.is_equal)
                pr = b // 2
                half = (b % 2) * 256
                if b % 2 == 0:
                    pv_tile = pv_ps.tile([128, 512], F32, tag="pv")
                    pv_tiles[pr] = pv_tile
                pv = pv_tiles[pr]
                out_sl = pv[:, half:half + 256]
                mms = [
                    (xt_tiles[tt][:, 0, off:off + 128], wct_sb[:, 0, :]),
                    (xt_tiles[tt][:, 1, off:off + 128], wct_sb[:, 1, :]),
                ]
                if has_b1:
                    mms.append((ones_sb[:, :], b1r_sb[:, :]))
                if b > 0:
                    mms.append((S[:, 0:128], st_tiles[b - 1][:, 0:256]))
                mms.append((S[:, 128:256], st_tiles[b][:, 0:256]))
                mms.append((S[:, 256:384], st_tiles[b][:, 256:512]))
                if b < nR - 1:
                    mms.append((S[:, 384:512], st_tiles[b + 1][:, 256:512]))
                for j, (lhsT, rhs) in enumerate(mms):
                    nc.tensor.matmul(out_sl, lhsT, rhs, start=(j == 0),
                                     stop=(j == len(mms) - 1))
                if b % 2 == 1:
                    vt = vtp.tile([128, 512], BF16, tag="vt")
                    vt_tiles[pr] = vt
                    nc.scalar.activation(vt[:, :], pv[:, :], AF.Prelu,
                                         alpha=SLOPE)
                if b % 4 == 3:
                    emit_quad(b // 4)

            def emit_quad(q):
                # transpose blocks 4q..4q+3 back to channel-major v tile
                v = vp.tile([128, 2, 516], BF16, tag="v")
                v_tiles[q] = v
                for cb in range(2):
                    pt = pt_ps.tile([128, 512], BF16, tag="pt")
                    for j4 in range(4):
                        vt = vt_tiles[2 * q + j4 // 2]
                        csl = (j4 % 2) * 256 + cb * 128
                        nc.tensor.transpose(pt[:, j4 * 128:(j4 + 1) * 128],
                                            vt[:, csl:csl + 128],
                                            ident_sb[:, :])
                    nc.scalar.activation(v[:, cb, 1:513], pt[:, :], AF.Copy)
                if q == 0:
                    nc.vector.memset(v[:, :, 0:1], 0.0)
                else:
                    nc.vector.tensor_copy(v[:, :, 0:1],
                                          v_tiles[q - 1][:, :, 512:513])
                    nc.vector.tensor_copy(v_tiles[q - 1][:, :, 513:514],
                                          v[:, :, 1:2])
                if q == nT - 1:
                    nc.vector.memset(v[:, :, 513:514], 0.0)
                if q >= 1:
                    emit_conv3(q - 1, wa_sb, b3_sb, i == NITER - 1,
                               head=(i == 0 and (q - 1) * 512 < HEAD))
                    if i + 1 < NITER:
                        xt_next[q - 1] = emit_A(q - 1)

            for b in range(nR + LAG):
                if i == 0 and b % 4 == 0:
                    tt = HEAD // 512 + b // 4
                    if tt < nT:
                        xt_tiles[tt] = emit_A(tt)
                if b < nR:
                    emit_B(b)
                if b >= LAG:
                    emit_G(b - LAG)
            emit_conv3(nT - 1, wa_sb, b3_sb, i == NITER - 1,
                       head=(i == 0 and (nT - 1) * 512 < HEAD))
            if i + 1 < NITER:
                xt_next[nT - 1] = emit_A(nT - 1)

    nc.compile()
    return nc


def _to_bf16(a):
    return np.asarray(a, dtype=np.float32).astype(ml_dtypes.bfloat16)


def prep_in_maps(x, d, WC, bC, WP, bP, WF, bF, WA, bA, T=T_FULL):
    """Build the 8 per-core input maps from the full-problem arrays.
    Returns (in_maps, has_b1)."""
    x = np.asarray(x, dtype=np.float32)
    d = np.asarray(d, dtype=np.float32)
    WC, WP, WF, WA = (np.asarray(w, dtype=np.float32) for w in (WC, WP, WF, WA))
    bC, bP, bF, bA = (np.asarray(b, dtype=np.float32) for b in (bC, bP, bF, bA))
    nb = x.shape[0]
    nR = T // 128

    wpf = np.empty((NITER, 2, 128, 512), np.float32)
    wct = np.empty((NITER, 2, 128, 256), np.float32)
    wa = np.empty((NITER, 3, 2, 2, 128, 128), np.float32)
    for i in range(NITER):
        wpfT = np.concatenate([WP[i].T, WF[i].T], axis=1)  # [c', 512]
        wpf[i] = wpfT.reshape(2, 128, 512)
        wct[i] = WC[i].T.reshape(2, 128, 256)              # [cc, p, o]
        for k in range(3):
            waT = WA[i, :, :, k].T                         # [c', o]
            wa[i, k] = waT.reshape(2, 128, 2, 128).transpose(0, 2, 1, 3)
    b1 = (bC + bP + bF).astype(np.float32)                  # [NITER, 256]
    has_b1 = bool(np.any(b1 != 0))
    b3 = bA.reshape(NITER, 2, 128, 1).astype(np.float32)

    wpf, wct, wa = _to_bf16(wpf), _to_bf16(wct), _to_bf16(wa)
    iota = np.arange(128, dtype=np.float32).reshape(128, 1)
    ident = _to_bf16(np.eye(128, dtype=np.float32))

    tf = np.arange(T, dtype=np.float32)
    in_maps = []
    for b in range(nb):
        dv = d[b, 0].astype(np.float32)
        ixr = np.full((NITER, nR, 512), INVALID, np.int16)
        for i, dil in enumerate(DILATIONS):
            dd = dv * np.float32(dil)
            rp = np.round(tf - dd).astype(np.int64)
            rf = np.round(tf + dd).astype(np.int64)
            rp = np.where(rp >= 0, rp, np.int64(-(1 << 32)))
            rf = np.where(rf < T, rf, np.int64(1 << 32))
            blk = np.arange(nR).repeat(128) * 128           # (T,)
            for c, base in enumerate((rp - blk + 128, rp - blk,
                                      rf - blk, rf - blk - 128)):
                v = np.where((base >= 0) & (base < 128), base,
                             np.int64(INVALID)).astype(np.int16)
                ixr[i, :, c * 128:(c + 1) * 128] = v.reshape(nR, 128)
        m = {
            "x": x[b].reshape(2, 128, T).copy(),
            "wpf": wpf, "wct": wct, "wa": wa, "b3": b3,
            "ixrel": ixr.astype(np.float32).astype(ml_dtypes.bfloat16),
            "iota": iota, "ident": ident,
        }
        if has_b1:
            m["b1row"] = _to_bf16(b1.reshape(NITER, 1, 256))
            m["ones"] = _to_bf16(np.ones((1, 128), np.float32))
        in_maps.append(m)
    return in_maps, has_b1


_nc_cache = {}


def kernel(**inputs) -> np.ndarray:
    T = inputs["x"].shape[2]
    in_maps, has_b1 = prep_in_maps(**inputs, T=T)
    key = (T, has_b1)
    if key not in _nc_cache:
        _nc_cache[key] = build_nc(T, has_b1=has_b1)
    nc = _nc_cache[key]
    res = run_bass_kernel_spmd(nc, in_maps, core_ids=list(range(8)))
    out = np.stack([np.asarray(res.results[i]["out"], dtype=np.float32)
                    .reshape(C, T) for i in range(8)])
    return out


# revision 21
# speedup vs baseline: 1.1672x; 1.1672x over previous
"""Trainium2 Bass kernel for nn_AdaptiveResBlock (8-core data-parallel).

Reference computation (per batch element b, C=256 channels, T=8192 time):
  for i, dil in enumerate((1, 2, 4)):
      xt = lrelu(x)
      xP, xF = time-gather of xt at round(t -/+ d*dil), zero out-of-range
      xt = WC@xt + WP@xP + WF@xF + biases        (1x1 convs over channels)
      xt = lrelu(xt)
      xt = conv3(xt, WA) + bias
      x = xt + x

Structure used:
  * The time-gather commutes with the 1x1 convs (indices are per-time,
    shared across channels):  WP @ gather(xt) == gather(WP @ xt).
  * Gather offsets are bounded by max(d)*dil <= 64 < 128, so for output
    token block b (128 tokens) the P-gather sources live in token blocks
    {b-1, b} and the F-gather sources in {b, b+1}.
  * The gather itself runs on the TensorEngine as one-hot matmuls in
    token-major space: S[j, t] = (idx(t) == j) built on DVE via is_equal
    against an iota column (relative indices DMA-broadcast from DRAM);
    out-of-range indices get no one-hot bit => free zero masking.
  * Per token block the PSUM accumulation fuses: uC^T (xt-stationary
    matmuls) + one-hot gathers of uP^T / uF^T (+ optional bias rank-1
    matmul).  ACT applies leaky-relu (Prelu, exact on HW) straight from
    PSUM; PE transposes v^T back to channel-major for the 3-tap conv.
  * The u^T token stripe is a rolling window of per-block SBUF tiles
    (B-stage runs a few blocks ahead of the gather stage).
  * lrelu(x) for the next iteration is emitted right behind each tile's
    residual update so the next iteration's matmuls start immediately.

Sharded data-parallel over B=8 across the 8 NeuronCores; (C,C) weights
replicated; per-core relative-gather-index tensors precomputed from d.
"""

import numpy as np
import ml_dtypes
from contextlib import ExitStack

import concourse.bass as bass
import concourse.tile as tile
from concourse import mybir, bacc
from concourse.bass_utils import run_bass_kernel_spmd

F32 = mybir.dt.float32
BF16 = mybir.dt.bfloat16
I16 = mybir.dt.int16
AF = mybir.ActivationFunctionType
OP = mybir.AluOpType

B, C, T_FULL = 8, 256, 8192
DILATIONS = (1, 2, 4)
NITER = len(DILATIONS)
SLOPE = 0.1
INVALID = -512
LAG = 3  # B-stage leads the gather stage by this many token blocks


def build_nc(T=T_FULL, num_devices=8, has_b1=False):
    nT = T // 512            # 512-wide time tiles
    nR = T // 128            # 128-wide token blocks

    nc = bacc.Bacc("TRN2", target_bir_lowering=False, debug=False,
                   num_devices=num_devices)
    x_d = nc.declare_dram_parameter("x", [2, 128, T], F32, isOutput=False)
    wpf_d = nc.declare_dram_parameter("wpf", [NITER, 2, 128, 512], BF16,
                                      isOutput=False)
    wct_d = nc.declare_dram_parameter("wct", [NITER, 2, 128, 256], BF16,
                                      isOutput=False)
    wa_d = nc.declare_dram_parameter("wa", [NITER, 3, 2, 2, 128, 128], BF16,
                                     isOutput=False)
    b3_d = nc.declare_dram_parameter("b3", [NITER, 2, 128, 1], F32,
                                     isOutput=False)
    ixr_d = nc.declare_dram_parameter("ixrel", [NITER, nR, 512], BF16,
                                      isOutput=False)
    iota_d = nc.declare_dram_parameter("iota", [128, 1], F32, isOutput=False)
    id_d = nc.declare_dram_parameter("ident", [128, 128], BF16, isOutput=False)
    if has_b1:
        b1r_d = nc.declare_dram_parameter("b1row", [NITER, 1, 256], BF16,
                                          isOutput=False)
        ones_d = nc.declare_dram_parameter("ones", [1, 128], BF16,
                                           isOutput=False)
    out_d = nc.declare_dram_parameter("out", [2, 128, T], F32, isOutput=True)

    with tile.TileContext(nc) as tc, ExitStack() as ctx:
        xpool = ctx.enter_context(tc.tile_pool(name="xres", bufs=1))
        stp = ctx.enter_context(tc.tile_pool(name="stp", bufs=LAG + 4))
        xtp = ctx.enter_context(tc.tile_pool(name="xtp", bufs=nT))
        vp = ctx.enter_context(tc.tile_pool(name="vp", bufs=nT))
        vtp = ctx.enter_context(tc.tile_pool(name="vtp", bufs=4))
        relp = ctx.enter_context(tc.tile_pool(name="relp", bufs=6))
        sp = ctx.enter_context(tc.tile_pool(name="sp", bufs=6))
        wts = ctx.enter_context(tc.tile_pool(name="wts", bufs=2))
        cst = ctx.enter_context(tc.tile_pool(name="cst", bufs=1))
        pu_ps = ctx.enter_context(tc.tile_pool(name="pu", bufs=2, space="PSUM"))
        pv_ps = ctx.enter_context(tc.tile_pool(name="pv", bufs=2, space="PSUM"))
        pt_ps = ctx.enter_context(tc.tile_pool(name="pt", bufs=4, space="PSUM"))

        def load_weights(i):
            wpf_sb = wts.tile([128, 2, 512], BF16, tag="wpf")
            for cc in range(2):
                nc.sync.dma_start(wpf_sb[:, cc, :], wpf_d[i, cc])
            wct_sb = wts.tile([128, 2, 256], BF16, tag="wct")
            for cc in range(2):
                nc.sync.dma_start(wct_sb[:, cc, :], wct_d[i, cc])
            wa_sb = wts.tile([128, 3, 2, 2, 128], BF16, tag="wa")
            for k in range(3):
                for cc in range(2):
                    for ob in range(2):
                        nc.sync.dma_start(wa_sb[:, k, cc, ob, :],
                                          wa_d[i, k, cc, ob])
            b3_sb = wts.tile([128, 2], F32, tag="b3")
            for ob in range(2):
                nc.sync.dma_start(b3_sb[:, ob:ob + 1], b3_d[i, ob])
            b1r_sb = None
            if has_b1:
                b1r_sb = wts.tile([1, 256], BF16, tag="b1r")
                nc.sync.dma_start(b1r_sb[:, :], b1r_d[i])
            return wpf_sb, wct_sb, wa_sb, b3_sb, b1r_sb

        # Head staging: the first columns of x land in a small dedicated
        # tile with an unambiguous 2-DMA dependency, so iteration 0 starts
        # without waiting for the bulk x load.  Iteration-0 weights go next
        # on the DMA queues, then the rest of x.
        HEAD = min(2048, T)
        x_head = cst.tile([128, 2, HEAD], F32)
        for cb in range(2):
            nc.sync.dma_start(x_head[:, cb, bass.ts(0, 1024)],
                              x_d[cb, :, bass.ts(0, 1024)])
        w0 = load_weights(0)
        for sg in range(1, HEAD // 1024):
            sl = bass.ts(sg, 1024)
            for cb in range(2):
                nc.sync.dma_start(x_head[:, cb, sl], x_d[cb, :, sl])
        iota_sb = cst.tile([128, 1], F32)
        nc.sync.dma_start(iota_sb[:, :], iota_d[:, :])
        ident_sb = cst.tile([128, 128], BF16)
        nc.sync.dma_start(ident_sb[:, :], id_d[:, :])
        if has_b1:
            ones_sb = cst.tile([1, 128], BF16)
            nc.sync.dma_start(ones_sb[:, :], ones_d[:, :])

        # Resident fp32 signal, [128 part, 2 channel-blocks, T].  Columns
        # [0, HEAD) are never read from x_sb in iteration 0 (x_head serves
        # them) and are fully written by iteration 0's residuals.  The bulk
        # load of the rest is emitted interleaved into iteration 0's block
        # loop so it shares the DMA ring fairly with the index broadcasts.
        x_sb = xpool.tile([128, 2, T], F32)
        for sg in range((T - HEAD) // 1024):
            sl = bass.ds(HEAD + sg * 1024, 1024)
            for cb in range(2):
                nc.sync.dma_start(x_sb[:, cb, sl], x_d[cb, :, sl])

        st_tiles = [None] * nR   # rolling u^T stripe tiles
        xt_tiles = [None] * nT
        xt_next = [None] * nT
        v_tiles = [None] * nT
        vt_tiles = [None] * (nR // 2)
        pv_tiles = [None] * (nR // 2)

        def emit_A(tt, head=False):
            tsl = bass.ts(tt, 512)
            src = x_head[:, :, tsl] if head else x_sb[:, :, tsl]
            xt = xtp.tile([128, 2, 512], BF16, tag="xt")
            nc.vector.scalar_tensor_tensor(
                xt[:, :, :], src, SLOPE, src, OP.mult, OP.max)
            return xt

        def emit_conv3(tt, wa_sb, b3_sb, last, head=False):
            tsl = bass.ts(tt, 512)
            for ob in range(2):
                py = pt_ps.tile([128, 512], F32, tag="pt")
                j = 0
                for k in range(3):
                    for cb in range(2):
                        nc.tensor.matmul(py[:, :], wa_sb[:, k, cb, ob, :],
                                         v_tiles[tt][:, cb, k:k + 512],
                                         start=(j == 0), stop=(j == 5))
                        j += 1
                # residual:  x = (y + b3) + x   (fp32; iter-0 head tiles
                # read the staging copy, write the resident tensor)
                xin = x_head[:, ob, tsl] if head else x_sb[:, ob, tsl]
                nc.vector.scalar_tensor_tensor(
                    x_sb[:, ob, tsl], py[:, :], b3_sb[:, ob:ob + 1],
                    xin, OP.add, OP.add)
            if last:
                for cb in range(2):
                    nc.sync.dma_start(out_d[cb, :, tsl], x_sb[:, cb, tsl])

        for i in range(NITER):
            wpf_sb, wct_sb, wa_sb, b3_sb, b1r_sb = \
                w0 if i == 0 else load_weights(i)

            if i == 0:
                for tt in range(nT):
                    xt_tiles[tt] = emit_A(tt, head=(tt * 512 < HEAD))
            else:
                xt_tiles, xt_next = xt_next, [None] * nT

            def emit_B(b):
                tt, off = b // 4, (b % 4) * 128
                ps = pu_ps.tile([128, 512], F32, tag="pu")
                nc.tensor.matmul(ps[:, :], xt_tiles[tt][:, 0, off:off + 128],
                                 wpf_sb[:, 0, :], start=True, stop=False)
                nc.tensor.matmul(ps[:, :], xt_tiles[tt][:, 1, off:off + 128],
                                 wpf_sb[:, 1, :], start=False, stop=True)
                st = stp.tile([128, 512], BF16, tag="st")
                st_tiles[b] = st
                nc.scalar.activation(st[:, :], ps[:, :], AF.Copy)

            def emit_G(b):
                tt, off = b // 4, (b % 4) * 128
                rel = relp.tile([128, 512], BF16, tag="rel")
                nc.sync.dma_start(rel[:, :],
                                  ixr_d[i, b].partition_broadcast(128))
                S = sp.tile([128, 512], BF16, tag="S")
                nc.vector.tensor_scalar(S[:, :], rel[:, :], iota_sb[:, 0:1],
                                        None, OP.is_equal)
                pr = b // 2
                half = (b % 2) * 256
                if b % 2 == 0:
                    pv_tile = pv_ps.tile([128, 512], F32, tag="pv")
                    pv_tiles[pr] = pv_tile
                pv = pv_tiles[pr]
                out_sl = pv[:, half:half + 256]
                mms = [
                    (xt_tiles[tt][:, 0, off:off + 128], wct_sb[:, 0, :]),
                    (xt_tiles[tt][:, 1, off:off + 128], wct_sb[:, 1, :]),
                ]
                if has_b1:
                    mms.append((ones_sb[:, :], b1r_sb[:, :]))
                if b > 0:
                    mms.append((S[:, 0:128], st_tiles[b - 1][:, 0:256]))
                mms.append((S[:, 128:256], st_tiles[b][:, 0:256]))
                mms.append((S[:, 256:384], st_tiles[b][:, 256:512]))
                if b < nR - 1:
                    mms.append((S[:, 384:512], st_tiles[b + 1][:, 256:512]))
                for j, (lhsT, rhs) in enumerate(mms):
                    nc.tensor.matmul(out_sl, lhsT, rhs, start=(j == 0),
                                     stop=(j == len(mms) - 1))
                if b % 2 == 1:
                    vt = vtp.tile([128, 512], BF16, tag="vt")
                    vt_tiles[pr] = vt
                    nc.scalar.activation(vt[:, :], pv[:, :], AF.Prelu,
                                         alpha=SLOPE)
                if b % 4 == 3:
                    emit_quad(b // 4)

            def emit_quad(q):
                # transpose blocks 4q..4q+3 back to channel-major v tile
                v = vp.tile([128, 2, 516], BF16, tag="v")
                v_tiles[q] = v
                for cb in range(2):
                    pt = pt_ps.tile([128, 512], BF16, tag="pt")
                    for j4 in range(4):
                        vt = vt_tiles[2 * q + j4 // 2]
                        csl = (j4 % 2) * 256 + cb * 128
                        nc.tensor.transpose(pt[:, j4 * 128:(j4 + 1) * 128],
                                            vt[:, csl:csl + 128],
                                            ident_sb[:, :])
                    nc.scalar.activation(v[:, cb, 1:513], pt[:, :], AF.Copy)
                if q == 0:
                    nc.vector.memset(v[:, :, 0:1], 0.0)
                else:
                    nc.vector.tensor_copy(v[:, :, 0:1],
                                          v_tiles[q - 1][:, :, 512:513])
                    nc.vector.tensor_copy(v_tiles[q - 1][:, :, 513:514],
                                          v[:, :, 1:2])
                if q == nT - 1:
                    nc.vector.memset(v[:, :, 513:514], 0.0)
                if q >= 1:
                    emit_conv3(q - 1, wa_sb, b3_sb, i == NITER - 1,
                               head=(i == 0 and (q - 1) * 512 < HEAD))
                    if i + 1 < NITER:
                        xt_next[q - 1] = emit_A(q - 1)

            for b in range(nR + LAG):
                if b < nR:
                    emit_B(b)
                if b >= LAG:
                    emit_G(b - LAG)
            emit_conv3(nT - 1, wa_sb, b3_sb, i == NITER - 1,
                       head=(i == 0 and (nT - 1) * 512 < HEAD))
            if i + 1 < NITER:
                xt_next[nT - 1] = emit_A(nT - 1)

    nc.compile()
    return nc


def _to_bf16(a):
    return np.asarray(a, dtype=np.float32).astype(ml_dtypes.bfloat16)


def prep_in_maps(x, d, WC, bC, WP, bP, WF, bF, WA, bA, T=T_FULL):
    """Build the 8 per-core input maps from the full-problem arrays.
    Returns (in_maps, has_b1)."""
    x = np.asarray(x, dtype=np.float32)
    d = np.asarray(d, dtype=np.float32)
    WC, WP, WF, WA = (np.asarray(w, dtype=np.float32) for w in (WC, WP, WF, WA))
    bC, bP, bF, bA = (np.asarray(b, dtype=np.float32) for b in (bC, bP, bF, bA))
    nb = x.shape[0]
    nR = T // 128

    wpf = np.empty((NITER, 2, 128, 512), np.float32)
    wct = np.empty((NITER, 2, 128, 256), np.float32)
    wa = np.empty((NITER, 3, 2, 2, 128, 128), np.float32)
    for i in range(NITER):
        wpfT = np.concatenate([WP[i].T, WF[i].T], axis=1)  # [c', 512]
        wpf[i] = wpfT.reshape(2, 128, 512)
        wct[i] = WC[i].T.reshape(2, 128, 256)              # [cc, p, o]
        for k in range(3):
            waT = WA[i, :, :, k].T                         # [c', o]
            wa[i, k] = waT.reshape(2, 128, 2, 128).transpose(0, 2, 1, 3)
    b1 = (bC + bP + bF).astype(np.float32)                  # [NITER, 256]
    has_b1 = bool(np.any(b1 != 0))
    b3 = bA.reshape(NITER, 2, 128, 1).astype(np.float32)

    wpf, wct, wa = _to_bf16(wpf), _to_bf16(wct), _to_bf16(wa)
    iota = np.arange(128, dtype=np.float32).reshape(128, 1)
    ident = _to_bf16(np.eye(128, dtype=np.float32))

    tf = np.arange(T, dtype=np.float32)
    in_maps = []
    for b in range(nb):
        dv = d[b, 0].astype(np.float32)
        ixr = np.full((NITER, nR, 512), INVALID, np.int16)
        for i, dil in enumerate(DILATIONS):
            dd = dv * np.float32(dil)
            rp = np.round(tf - dd).astype(np.int64)
            rf = np.round(tf + dd).astype(np.int64)
            rp = np.where(rp >= 0, rp, np.int64(-(1 << 32)))
            rf = np.where(rf < T, rf, np.int64(1 << 32))
            blk = np.arange(nR).repeat(128) * 128           # (T,)
            for c, base in enumerate((rp - blk + 128, rp - blk,
                                      rf - blk, rf - blk - 128)):
                v = np.where((base >= 0) & (base < 128), base,
                             np.int64(INVALID)).astype(np.int16)
                ixr[i, :, c * 128:(c + 1) * 128] = v.reshape(nR, 128)
        m = {
            "x": x[b].reshape(2, 128, T).copy(),
            "wpf": wpf, "wct": wct, "wa": wa, "b3": b3,
            "ixrel": ixr.astype(np.float32).astype(ml_dtypes.bfloat16),
            "iota": iota, "ident": ident,
        }
        if has_b1:
            m["b1row"] = _to_bf16(b1.reshape(NITER, 1, 256))
            m["ones"] = _to_bf16(np.ones((1, 128), np.float32))
        in_maps.append(m)
    return in_maps, has_b1


_nc_cache = {}


def kernel(**inputs) -> np.ndarray:
    T = inputs["x"].shape[2]
    in_maps, has_b1 = prep_in_maps(**inputs, T=T)
    key = (T, has_b1)
    if key not in _nc_cache:
        _nc_cache[key] = build_nc(T, has_b1=has_b1)
    nc = _nc_cache[key]
    res = run_bass_kernel_spmd(nc, in_maps, core_ids=list(range(8)))
    out = np.stack([np.asarray(res.results[i]["out"], dtype=np.float32)
                    .reshape(C, T) for i in range(8)])
    return out
